# revision 19
# baseline (speedup 1.0000x reference)
"""AlchemicalGAT Trainium2 kernel (8 NeuronCores, SPMD).

Strategy:
  - Shard dst nodes contiguously across 8 cores (2500 each); edges sharded by dst.
  - Per layer, each core builds a packed per-node staging row in SBUF:
    [h (4x64 fp8e4m3, 256B) | esrc hi/lo (bf16x4 each) | edst hi/lo] = 288B.
    An SBUF->SBUF AllGather exchanges all cores' staging (8x20 blocks); the
    result is copied out to a DRAM gather table with 512B row pitch.
  - Edge phase per dst block: one dma_gather of 512B rows by src (h+esrc),
    one 256B-at-offset-256B gather by dst (edst); softmax without
    max-subtraction; segment-sum via one-hot matmul into PSUM.
  - LayerNorm folded into the conv1 matmul: x^T arrives via DMA transpose,
    mean/sumsq come from ones-matmuls, and [mu; 1/r] form two extra lhsT rows
    against host-built correction weights; the result is scaled by r per node.
  - Node transforms/MLP batch channel pairs via block-diagonal bf16 weights.
  - Per-structure energies via matmul-masked segment sum; host sums 8 cores.
"""
import sys, os
sys.path.insert(0, "/opt/trn_rl_repo")
import math
import numpy as np
import ml_dtypes
from contextlib import ExitStack

import concourse.bass as bass
import concourse.bacc as bacc
import concourse.mybir as mybir
import concourse.tile as tile
from concourse.bass_utils import run_bass_kernel_spmd
from concourse.masks import make_identity

F32 = mybir.dt.float32
BF16 = mybir.dt.bfloat16
U8 = mybir.dt.uint8
FP8 = mybir.dt.float8e4
I16 = mybir.dt.int16
AF = mybir.ActivationFunctionType
ALU = mybir.AluOpType

N = 20000
E = 400000
C = 4
F = 128
O = 64          # conv1/conv2 width
H1, H2 = 64, 32
S = 32          # structures
NCORE = 8
NPC = N // NCORE            # 2500 nodes per core
BLK = 128
NBLK = math.ceil(NPC / BLK)  # 20 (last block 68 nodes)
LAST = NPC - (NBLK - 1) * BLK  # 68
SLOT = NBLK * BLK            # 2560 table rows per core slot
SROW = 288                   # staging row bytes (h fp8 256 + 4x bf16x4)
TROW = 512                   # DRAM table row pitch bytes
NEG = -1.0e30

GCHUNK = int(os.environ.get("GAT_GCHUNK", "8"))  # gather chunk (tiles; 1024 rows = SWDGE ring capacity)


# ---------------------------------------------------------------- host side

def _wrap_idx16(ids):
    """[n] int -> dma_gather index layout [128, n//16] int16 (q -> [q%16, q//16],
    replicated over the 8 Q7 cores)."""
    n = ids.shape[0]
    assert n % 16 == 0
    out = np.zeros((16, n // 16), np.int16)
    q = np.arange(n)
    out[q % 16, q // 16] = ids.astype(np.int16)
    return np.tile(out, (8, 1))


def _prep(inputs):
    x = np.asarray(inputs["x"], dtype=np.float32)
    ei = np.asarray(inputs["edge_index"]).astype(np.int64)
    bid = np.asarray(inputs["batch_ids"]).astype(np.int64)
    gamma = np.asarray(inputs["gamma"], np.float32)
    beta = np.asarray(inputs["beta"], np.float32)
    src, dst = ei[0], ei[1]

    order = np.argsort(dst, kind="stable")
    src_s, dst_s = src[order], dst[order]

    counts = np.bincount(dst_s, minlength=N)
    starts = np.concatenate([[0], np.cumsum(counts)])
    blk_cnt = np.zeros((NCORE, NBLK), np.int64)
    for c in range(NCORE):
        for b in range(NBLK):
            lo = c * NPC + b * BLK
            hi = min(c * NPC + min((b + 1) * BLK, NPC), (c + 1) * NPC)
            blk_cnt[c, b] = starts[hi] - starts[lo]
    # per-block tile count, max over cores (SPMD uniform program)
    tbs = tuple(int(math.ceil(blk_cnt[:, b].max() / BLK)) for b in range(NBLK))
    offs = np.concatenate([[0], np.cumsum(tbs)]).astype(np.int64)
    TT = int(offs[-1])

    # table row index: node n lives at (n//NPC)*SLOT + n%NPC; row NPC of
    # core 0's slot is the pad row.
    def tidx(n):
        return (n // NPC) * SLOT + n % NPC

    per_core = []
    for c in range(NCORE):
        s16 = np.zeros((128, TT * 8), np.int16)
        d16 = np.zeros((128, TT * 8), np.int16)
        dl = np.full((128, TT, 1), 127.0, np.float32)
        for b in range(NBLK):
            EB = tbs[b] * BLK
            src_ids = np.full(EB, NPC, np.int64)   # pad -> core0 pad row
            dst_ids = np.full(EB, NPC, np.int64)
            dloc = np.full(EB, 127, np.int64)
            lo = c * NPC + b * BLK
            hi = min(c * NPC + min((b + 1) * BLK, NPC), (c + 1) * NPC)
            e0, e1 = starts[lo], starts[hi]
            n = e1 - e0
            src_ids[:n] = tidx(src_s[e0:e1])
            dst_ids[:n] = tidx(dst_s[e0:e1])
            dloc[:n] = dst_s[e0:e1] - lo
            s16[:, offs[b] * 8:offs[b + 1] * 8] = _wrap_idx16(src_ids)
            d16[:, offs[b] * 8:offs[b + 1] * 8] = _wrap_idx16(dst_ids)
            dl[:, offs[b]:offs[b + 1], 0] = dloc.reshape(tbs[b], 128).T
        # structure mask [128, NBLK*S]
        bm = np.zeros((128, NBLK * S), np.float32)
        for b in range(NBLK):
            cnt = BLK if b < NBLK - 1 else LAST
            g = c * NPC + b * BLK + np.arange(cnt)
            bm[np.arange(cnt), b * S + bid[g]] = 1.0
        xs = np.zeros((NBLK * BLK, C * F), np.float32)
        xs[:NPC] = x[c * NPC:(c + 1) * NPC].reshape(NPC, C * F)
        per_core.append(dict(
            xs=xs.astype(ml_dtypes.bfloat16),
            src16=s16, dst16=d16,
            dstloc=dl.astype(ml_dtypes.bfloat16),
            bmask=bm.astype(ml_dtypes.bfloat16)))

    # ---- weights
    bg = beta / np.where(gamma == 0, 1.0, gamma)

    def wcat(W, a_s, a_d):
        out = np.zeros((C, W.shape[1], 66), np.float32)
        out[:, :, :O] = W
        out[:, :, O] = np.einsum("cfo,co->cf", W, a_s)
        out[:, :, O + 1] = np.einsum("cfo,co->cf", W, a_d)
        return out

    Wc1 = np.asarray(inputs["Wc1"], np.float32) * gamma[None, :, None]
    wcat1 = wcat(Wc1, np.asarray(inputs["asrc1"], np.float32),
                 np.asarray(inputs["adst1"], np.float32))       # [C,F,66]
    wcat2 = wcat(np.asarray(inputs["Wc2"], np.float32),
                 np.asarray(inputs["asrc2"], np.float32),
                 np.asarray(inputs["adst2"], np.float32))       # [C,O,66]

    # conv1 rhs [F, C*66]
    w1 = np.zeros((F, C * 66), np.float32)
    for c in range(C):
        w1[:, c * 66:(c + 1) * 66] = wcat1[c]
    # LN-correction rhs [8, C*66]: row c = -colsum_f(wcat1_c); row 4+c =
    # sum_f bg[f]*wcat1_c[f,:]
    rhs2 = np.zeros((8, C * 66), np.float32)
    for c in range(C):
        rhs2[c, c * 66:(c + 1) * 66] = -wcat1[c].sum(axis=0)
        rhs2[4 + c, c * 66:(c + 1) * 66] = bg @ wcat1[c]

    # conv2 block-diag pairs [O*2=128, 2*132]
    w2bd = np.zeros((128, 2 * 132), np.float32)
    for p in range(2):
        for j in range(2):
            c = 2 * p + j
            w2bd[j * O:(j + 1) * O, p * 132 + j * 66:p * 132 + (j + 1) * 66] = wcat2[c]

    Wn1 = np.asarray(inputs["Wn1"], np.float32)
    Wn2 = np.asarray(inputs["Wn2"], np.float32)
    Wout = np.asarray(inputs["Wout"], np.float32) / np.float32(math.sqrt(C) * 20.0)
    wn1bd = np.zeros((128, 2 * 128), np.float32)
    wn2bd = np.zeros((128, 2 * 64), np.float32)
    for p in range(2):
        for j in range(2):
            c = 2 * p + j
            wn1bd[j * O:(j + 1) * O, p * 128 + j * O:p * 128 + (j + 1) * O] = Wn1[c]
            wn2bd[j * O:(j + 1) * O, p * 64 + j * H2:p * 64 + (j + 1) * H2] = Wn2[c]
    wof = np.zeros((128, 1), np.float32)
    for c in range(C):
        wof[c * H2:(c + 1) * H2, 0] = Wout[c, :, 0]

    iota = np.tile(np.arange(128, dtype=np.float32), (128, 1)).reshape(128, 1, 128)
    oneF = np.full((128, 1), 1.0 / F, np.float32)
    padmask = np.ones((128, 1), np.float32); padmask[LAST, 0] = 0.0
    padneg = np.zeros((128, 1), np.float32); padneg[LAST, 0] = NEG

    b16 = lambda a: np.asarray(a, np.float32).astype(ml_dtypes.bfloat16)
    shared = dict(
        w1=b16(w1), rhs2=b16(rhs2), w2bd=b16(w2bd),
        wn1bd=b16(wn1bd), wn2bd=b16(wn2bd), wof=b16(wof),
        iota=b16(iota), oneF=b16(oneF),
        padmask=padmask, padneg=padneg,
    )
    return per_core, shared, tbs


# ---------------------------------------------------------------- device side

STOP_AFTER = os.environ.get("GAT_STOP_AFTER", "")  # node1|ag1|edge1|node2|edge2


def _build(tbs):
    nc = bacc.Bacc("TRN2", target_bir_lowering=False, debug=False,
                   enable_asserts=False, num_devices=NCORE)
    tbs = list(tbs)
    tmax = max(tbs)
    offs = [0]
    for t in tbs:
        offs.append(offs[-1] + t)
    TT = offs[-1]

    xs_d = nc.dram_tensor("xs", [NBLK * BLK, C * F], BF16, kind="ExternalInput")
    s16_d = nc.dram_tensor("src16", [128, TT * 8], I16, kind="ExternalInput")
    d16_d = nc.dram_tensor("dst16", [128, TT * 8], I16, kind="ExternalInput")
    dl_d = nc.dram_tensor("dstloc", [128, TT, 1], BF16, kind="ExternalInput")
    bm_d = nc.dram_tensor("bmask", [128, NBLK * S], BF16, kind="ExternalInput")
    w1_d = nc.dram_tensor("w1", [F, C * 66], BF16, kind="ExternalInput")
    rhs2_d = nc.dram_tensor("rhs2", [8, C * 66], BF16, kind="ExternalInput")
    w2bd_d = nc.dram_tensor("w2bd", [128, 2 * 132], BF16, kind="ExternalInput")
    wn1bd_d = nc.dram_tensor("wn1bd", [128, 2 * 128], BF16, kind="ExternalInput")
    wn2bd_d = nc.dram_tensor("wn2bd", [128, 2 * 64], BF16, kind="ExternalInput")
    wof_d = nc.dram_tensor("wof", [128, 1], BF16, kind="ExternalInput")
    io_d = nc.dram_tensor("iota", [128, 1, 128], BF16, kind="ExternalInput")
    one_d = nc.dram_tensor("oneF", [128, 1], BF16, kind="ExternalInput")
    pm_d = nc.dram_tensor("padmask", [128, 1], F32, kind="ExternalInput")
    pn_d = nc.dram_tensor("padneg", [128, 1], F32, kind="ExternalInput")
    out_d = nc.dram_tensor("energy", [S, 1], F32, kind="ExternalOutput")

    with tile.TileContext(nc, num_cores=NCORE) as tc, ExitStack() as ctx:
        per = ctx.enter_context(tc.tile_pool(name="persist", bufs=1))
        sbw = ctx.enter_context(tc.tile_pool(
            name="work", bufs=int(os.environ.get("GAT_SBW_BUFS", "2"))))
        sbn = ctx.enter_context(tc.tile_pool(
            name="nwork", bufs=int(os.environ.get("GAT_SBN_BUFS", "3"))))
        ps_node = ctx.enter_context(tc.tile_pool(
            name="psn", bufs=int(os.environ.get("GAT_PSN_BUFS", "2")), space="PSUM"))
        ps_tp = ctx.enter_context(tc.tile_pool(name="pst", bufs=2, space="PSUM"))
        ps_edge = ctx.enter_context(tc.tile_pool(
            name="pse", bufs=int(os.environ.get("GAT_PSE_BUFS", "2")), space="PSUM"))
        dram = ctx.enter_context(tc.tile_pool(name="dram", bufs=1, space="DRAM"))

        # persistent tiles
        s16 = per.tile([128, TT * 8], I16)
        d16 = per.tile([128, TT * 8], I16)
        dl = per.tile([128, TT, 1], BF16)
        bm = per.tile([128, NBLK * S], BF16)
        iot = per.tile([128, 1, 128], BF16)
        oneF = per.tile([128, 1], BF16)
        w1 = per.tile([F, C * 66], BF16)
        rhs2 = per.tile([8, C * 66], BF16)
        w2bd = per.tile([128, 2 * 132], BF16)
        wn1bd = per.tile([128, 2 * 128], BF16)
        wn2bd = per.tile([128, 2 * 64], BF16)
        wof = per.tile([128, 1], BF16)
        pmsk = per.tile([128, 1], F32)
        pneg = per.tile([128, 1], F32)
        ident = per.tile([128, 128], BF16)
        stg = per.tile([128, NBLK * SROW], U8)
        Hb = per.tile([128, NBLK, C * O], BF16)
        H3 = per.tile([128, NBLK, C * O], BF16)
        Eb = per.tile([128, NBLK], BF16)
        eps = per.tile([128, 1], F32)
        nc.vector.memset(eps[:], 1e-5)

        nc.sync.dma_start(s16[:], s16_d[:, :])
        nc.sync.dma_start(d16[:], d16_d[:, :])
        nc.sync.dma_start(dl[:], dl_d[:, :, :])
        nc.sync.dma_start(bm[:], bm_d[:, :])
        nc.sync.dma_start(iot[:], io_d[:, :, :])
        nc.sync.dma_start(oneF[:], one_d[:, :])
        nc.sync.dma_start(w1[:], w1_d[:, :])
        nc.sync.dma_start(rhs2[:], rhs2_d[:, :])
        nc.sync.dma_start(w2bd[:], w2bd_d[:, :])
        nc.sync.dma_start(wn1bd[:], wn1bd_d[:, :])
        nc.sync.dma_start(wn2bd[:], wn2bd_d[:, :])
        nc.sync.dma_start(wof[:], wof_d[:, :])
        nc.sync.dma_start(pmsk[:], pm_d[:, :])
        nc.sync.dma_start(pneg[:], pn_d[:, :])
        make_identity(nc, ident[:])

        # DRAM: packed staging rows, packed AllGather output, and the
        # 512B-pitch gather tables
        tbsrc = [dram.tile([SLOT, SROW], U8, name=f"tbsrc{i}") for i in range(2)]
        agd = [dram.tile([NCORE * SLOT, SROW], U8, name=f"agd{i}") for i in range(2)]
        tbl = [dram.tile([NCORE * SLOT, TROW], U8, name=f"tbl{i}") for i in range(2)]

        def silu_to(pool, out_ap, in_ap, shape, tag, sb_in=None):
            """out = in * sigmoid(in) via tanh (ACT exp-set friendly).
            If sb_in (bf16 SBUF copy of in) is given, the final multiply runs
            all-bf16 for 2x DVE throughput."""
            th = pool.tile(shape, BF16, tag=tag)
            nc.scalar.activation(th[:], in_ap, AF.Tanh, scale=0.5)
            nc.vector.tensor_scalar(out=th[:], in0=th[:], scalar1=0.5, scalar2=0.5,
                                    op0=ALU.mult, op1=ALU.add)
            nc.vector.tensor_tensor(out=out_ap, in0=sb_in if sb_in is not None
                                    else in_ap, in1=th[:], op=ALU.mult)

        def stg_views(nt):
            blk = stg[:, nt * SROW:(nt + 1) * SROW]
            h = blk[:, 0:256].bitcast(FP8).rearrange("p (c u) -> p c u", c=C)
            es_hi = blk[:, 256:264].bitcast(BF16)
            es_lo = blk[:, 264:272].bitcast(BF16)
            ed_hi = blk[:, 272:280].bitcast(BF16)
            ed_lo = blk[:, 280:288].bitcast(BF16)
            return h, es_hi, es_lo, ed_hi, ed_lo

        def pad_row_fix(nt):
            # pad row (local row NPC = LAST within the last block): force
            # esrc_hi = NEG on that row via full-partition masked ops so pad
            # edges get ex = 0 (partition-offset writes are not allowed).
            _, es_hi, _, _, _ = stg_views(nt)
            nc.vector.tensor_scalar(out=es_hi[:], in0=es_hi[:],
                                    scalar1=pmsk[:], scalar2=None, op0=ALU.mult)
            nc.vector.tensor_scalar(out=es_hi[:], in0=es_hi[:],
                                    scalar1=pneg[:], scalar2=None, op0=ALU.add)

        def assemble(nt, hsrc, esrc_f32, edst_f32):
            """hsrc [128,C,64] -> fp8; esrc/edst f32 [128,4] -> bf16 hi/lo."""
            h, es_hi, es_lo, ed_hi, ed_lo = stg_views(nt)
            nc.vector.tensor_copy(h[:], hsrc)
            nc.vector.tensor_copy(es_hi[:], esrc_f32)
            nc.vector.tensor_tensor(out=es_lo[:], in0=esrc_f32, in1=es_hi[:],
                                    op=ALU.subtract)
            nc.vector.tensor_copy(ed_hi[:], edst_f32)
            nc.vector.tensor_tensor(out=ed_lo[:], in0=edst_f32, in1=ed_hi[:],
                                    op=ALU.subtract)
            if nt == NBLK - 1:
                pad_row_fix(nt)

        def exchange(layer):
            nc.sync.dma_start(
                tbsrc[layer][:, :].rearrange("(b p) c -> p b c", p=128),
                stg[:, :].rearrange("p (b c) -> p b c", b=NBLK))
            nc.gpsimd.collective_compute(
                "AllGather", ALU.bypass, replica_groups=[list(range(NCORE))],
                ins=[tbsrc[layer][:, :]], outs=[agd[layer][:, :]])
            nc.sync.dma_start(tbl[layer][:, 0:SROW], agd[layer][:, :])

        def node_phase1(do_ag=True):
            for nt in range(NBLK):
                # x^T (all channels) via one DMA transpose: [128 f, C, 128 n]
                xT = sbn.tile([128, C, 128], BF16, tag="xT")
                nc.sync.dma_start_transpose(
                    xT[:], xs_d[nt * BLK:(nt + 1) * BLK, :])
                xsq = sbn.tile([128, C, 128], BF16, tag="xsq")
                nc.scalar.activation(xsq[:], xT[:], AF.Square)
                # stats: mu_c, sumsq_c/F via ones-matmuls -> [128 nodes, 8]
                stp = ps_tp.tile([128, 8], F32, space="PSUM", tag="tp")
                for cc in range(C):
                    nc.tensor.matmul(out=stp[:, cc:cc + 1],
                                     lhsT=xT[:, cc, :],
                                     rhs=oneF[:, :], start=True, stop=True)
                    nc.tensor.matmul(out=stp[:, 4 + cc:5 + cc],
                                     lhsT=xsq[:, cc, :],
                                     rhs=oneF[:, :], start=True, stop=True)
                sb8 = sbn.tile([128, 8], BF16, tag="sb8")
                nc.vector.tensor_copy(sb8[:, 0:4], stp[:, 0:4])
                mu2 = sbn.tile([128, 4], F32, tag="mu2")
                nc.vector.tensor_tensor(out=mu2[:], in0=sb8[:, 0:4],
                                        in1=sb8[:, 0:4], op=ALU.mult)
                var = sbn.tile([128, 4], F32, tag="var")
                nc.vector.tensor_tensor(out=var[:], in0=stp[:, 4:8],
                                        in1=mu2[:], op=ALU.subtract)
                invr = sbn.tile([128, 4], F32, tag="invr")
                nc.scalar.activation(invr[:], var[:], AF.Sqrt, bias=eps[:])
                rr = sbn.tile([128, 4], F32, tag="rr")
                nc.vector.reciprocal(rr[:], invr[:])
                nc.vector.tensor_copy(sb8[:, 4:8], invr[:])
                # [mu | invr] -> transpose -> [8, 128] lhsT rows
                tp8 = ps_tp.tile([8, 128], BF16, space="PSUM", tag="tp")
                nc.tensor.transpose(out=tp8[:], in_=sb8[:], identity=ident[:])
                st8 = sbn.tile([8, 128], BF16, tag="st8")
                nc.vector.tensor_copy(st8[:], tp8[:])
                # conv1 matmul + LN correction rows, then scale by r
                nps = ps_node.tile([128, C * 66], F32, space="PSUM", tag="nps")
                for cc in range(C):
                    nc.tensor.matmul(out=nps[:, cc * 66:(cc + 1) * 66],
                                     lhsT=xT[:, cc, :],
                                     rhs=w1[:, cc * 66:(cc + 1) * 66],
                                     start=True, stop=False)
                    nc.tensor.matmul(out=nps[:, cc * 66:(cc + 1) * 66],
                                     lhsT=st8[:, :],
                                     rhs=rhs2[:, cc * 66:(cc + 1) * 66],
                                     start=False, stop=True)
                ht = sbn.tile([128, C * 66], F32, tag="ht")
                for cc in range(C):
                    nc.vector.tensor_scalar_mul(
                        ht[:, cc * 66:(cc + 1) * 66],
                        nps[:, cc * 66:(cc + 1) * 66], rr[:, cc:cc + 1])
                htv = ht[:, :].rearrange("p (c u) -> p c u", c=C)
                assemble(nt, htv[:, :, 0:64], htv[:, :, 64], htv[:, :, 65])
            if do_ag:
                exchange(0)

        def node_phase2(do_ag=True):
            for nt in range(NBLK):
                nps = ps_node.tile([128, 264], F32, space="PSUM", tag="nps")
                hT = sbn.tile([128, 2, 128], BF16, tag="hT")
                nc.sync.dma_start_transpose(hT[:], Hb[:, nt, :])
                for p in range(2):
                    nc.tensor.matmul(out=nps[:, p * 132:(p + 1) * 132],
                                     lhsT=hT[:, p, :],
                                     rhs=w2bd[:, p * 132:(p + 1) * 132],
                                     start=True, stop=True)
                h, es_hi, es_lo, ed_hi, ed_lo = stg_views(nt)
                es = sbn.tile([128, 4], F32, tag="es")
                ed = sbn.tile([128, 4], F32, tag="ed")
                for p in range(2):
                    v = nps[:, p * 132:(p + 1) * 132].rearrange(
                        "p (c u) -> p c u", c=2)
                    nc.vector.tensor_copy(h[:, 2 * p:2 * p + 2, :], v[:, :, 0:64])
                    nc.vector.tensor_copy(es[:, 2 * p:2 * p + 2], v[:, :, 64])
                    nc.vector.tensor_copy(ed[:, 2 * p:2 * p + 2], v[:, :, 65])
                nc.vector.tensor_copy(es_hi[:], es[:])
                nc.vector.tensor_tensor(out=es_lo[:], in0=es[:], in1=es_hi[:],
                                        op=ALU.subtract)
                nc.vector.tensor_copy(ed_hi[:], ed[:])
                nc.vector.tensor_tensor(out=ed_lo[:], in0=ed[:], in1=ed_hi[:],
                                        op=ALU.subtract)
                if nt == NBLK - 1:
                    pad_row_fix(nt)
            if do_ag:
                exchange(1)

        def edge_phase(layer, Hout):
            for b in range(NBLK):
                tb = tbs[b]
                G = sbw.tile([128, tmax, TROW], U8, tag="G")
                D = sbw.tile([128, tmax, 256], U8, tag="D")
                for t0 in range(0, tb, GCHUNK):
                    t1 = min(t0 + GCHUNK, tb)
                    k = (t1 - t0) * BLK
                    o0 = (offs[b] + t0) * 8
                    o1 = (offs[b] + t1) * 8
                    nc.gpsimd.dma_gather(G[:, t0:t1, :], tbl[layer][:, :],
                                         s16[:, o0:o1], k, k, TROW)
                    nc.gpsimd.dma_gather(D[:, t0:t1, :], tbl[layer][:, 256:TROW],
                                         d16[:, o0:o1], k, k, 256, elem_step=TROW)
                OS = sbw.tile([128, tmax, 128], BF16, tag="OS")
                nc.vector.tensor_tensor(
                    out=OS[:, 0:tb],
                    in0=dl[:, offs[b]:offs[b] + tb, :].to_broadcast([128, tb, 128]),
                    in1=iot[:, :, :].to_broadcast([128, tb, 128]),
                    op=ALU.is_equal)
                gs_hi = G[:, :, 256:264].bitcast(BF16)
                gs_lo = G[:, :, 264:272].bitcast(BF16)
                dd_hi = D[:, :, 16:24].bitcast(BF16)
                dd_lo = D[:, :, 24:32].bitcast(BF16)
                EX = sbw.tile([128, tmax, C], F32, tag="EX")
                nc.vector.tensor_tensor(out=EX[:, 0:tb], in0=gs_hi[:, 0:tb],
                                        in1=gs_lo[:, 0:tb], op=ALU.add)
                nc.vector.tensor_tensor(out=EX[:, 0:tb], in0=EX[:, 0:tb],
                                        in1=dd_hi[:, 0:tb], op=ALU.add)
                nc.vector.tensor_tensor(out=EX[:, 0:tb], in0=EX[:, 0:tb],
                                        in1=dd_lo[:, 0:tb], op=ALU.add)
                nc.vector.scalar_tensor_tensor(
                    out=EX[:, 0:tb], in0=EX[:, 0:tb], scalar=0.2,
                    in1=EX[:, 0:tb], op0=ALU.mult, op1=ALU.max)
                nc.scalar.activation(EX[:, 0:tb], EX[:, 0:tb], AF.Exp)
                Gh = G[:, :, 0:256].bitcast(FP8).rearrange(
                    "p t (c u) -> p t c u", c=C)
                Gw = sbw.tile([128, tmax, 260], BF16, tag="Gw")
                Gwv = Gw[:, :, 0:256].rearrange("p t (c u) -> p t c u", c=C)
                nc.vector.tensor_tensor(
                    out=Gwv[:, 0:tb], in0=Gh[:, 0:tb],
                    in1=EX[:, 0:tb].to_broadcast([128, tb, C, 64]), op=ALU.mult)
                nc.vector.tensor_copy(Gw[:, 0:tb, 256:260], EX[:, 0:tb])
                ps = ps_edge.tile([128, 260], F32, space="PSUM", tag="ep")
                for t in range(tb):
                    nc.tensor.matmul(out=ps[:], lhsT=OS[:, t, :], rhs=Gw[:, t, :],
                                     start=(t == 0), stop=(t == tb - 1))
                dn = sbw.tile([128, C], F32, tag="dn")
                nc.vector.tensor_scalar(out=dn[:], in0=ps[:, 256:260], scalar1=1e-16,
                                        scalar2=None, op0=ALU.add)
                rc = sbw.tile([128, C], F32, tag="rc")
                nc.vector.reciprocal(rc[:], dn[:])
                om = sbw.tile([128, C * O], BF16, tag="om")
                omv = om[:, :].rearrange("p (c u) -> p c u", c=C)
                psv = ps[:, 0:256].rearrange("p (c u) -> p c u", c=C)
                nc.vector.tensor_tensor(out=omv[:, :, :], in0=psv[:, :, :],
                                        in1=rc[:].to_broadcast([128, C, 64]),
                                        op=ALU.mult)
                silu_to(sbw, Hout[:, b, :], om[:], [128, C * O], "th",
                        sb_in=om[:])

        phases = {"node1": 1, "ag1": 2, "edge1": 3, "node2": 4, "edge2": 5}
        stop = phases.get(STOP_AFTER, 99)

        node_phase1(do_ag=(stop >= 2))
        if stop >= 3:
            edge_phase(0, Hb)
        if stop >= 4:
            node_phase2(do_ag=(stop >= 5))
        if stop >= 5:
            edge_phase(1, H3)
        do_tail = stop >= 6
        if not do_tail:
            eo0 = sbn.tile([S, 1], F32, tag="eo")
            nc.vector.memset(eo0[:], 0.0)
            nc.sync.dma_start(out_d[:, :], eo0[:])

        # MLP + channel sum + structure segment sum
        for nt in (range(NBLK) if do_tail else []):
            u2st = sbn.tile([128, 128], BF16, tag="u2st")
            p1 = ps_node.tile([128, 264], F32, space="PSUM", tag="nps")
            hT = sbn.tile([128, 2, 128], BF16, tag="tT")
            nc.sync.dma_start_transpose(hT[:], H3[:, nt, :])
            for p in range(2):
                nc.tensor.matmul(out=p1[:, p * 128:(p + 1) * 128],
                                 lhsT=wn1bd[:, p * 128:(p + 1) * 128],
                                 rhs=hT[:, p, :], start=True, stop=True)
            for p in range(2):
                u1 = sbn.tile([128, 128], BF16, tag=f"u1{p}")
                silu_to(sbn, u1[:], p1[:, p * 128:(p + 1) * 128], [128, 128],
                        f"th1{p}")
                p2 = ps_edge.tile([64, 128], F32, space="PSUM", tag="ep")
                nc.tensor.matmul(out=p2[:], lhsT=wn2bd[:, p * 64:(p + 1) * 64],
                                 rhs=u1[:], start=True, stop=True)
                silu_to(sbn, u2st[p * 64:(p + 1) * 64, :], p2[:], [64, 128],
                        f"th2{p}")
            p3 = ps_edge.tile([128, 1], F32, space="PSUM", tag="ep")
            nc.tensor.matmul(out=p3[:], lhsT=u2st[:], rhs=wof[:, :],
                             start=True, stop=True)
            nc.vector.tensor_copy(Eb[:, nt:nt + 1], p3[:])
        if do_tail:
            psS = ps_tp.tile([S, 1], F32, space="PSUM", tag="tp")
            for nt in range(NBLK):
                nc.tensor.matmul(out=psS[:], lhsT=bm[:, nt * S:(nt + 1) * S],
                                 rhs=Eb[:, nt:nt + 1],
                                 start=(nt == 0), stop=(nt == NBLK - 1))
            eo = sbn.tile([S, 1], F32, tag="eo")
            nc.vector.tensor_copy(eo[:], psS[:])
            nc.sync.dma_start(out_d[:, :], eo[:])
    nc.compile()
    return nc


_CACHE = {}


def kernel(**inputs):
    per_core, shared, tbs = _prep(inputs)
    if tbs not in _CACHE:
        _CACHE[tbs] = _build(tbs)
    nc = _CACHE[tbs]
    in_maps = []
    for c in range(NCORE):
        pc = per_core[c]
        m = {"xs": pc["xs"], "src16": pc["src16"], "dst16": pc["dst16"],
             "dstloc": pc["dstloc"], "bmask": pc["bmask"]}
        m.update(shared)
        in_maps.append(m)
    res = run_bass_kernel_spmd(nc, in_maps, core_ids=list(range(NCORE)))
    out = np.zeros((S, 1), np.float32)
    for c in range(NCORE):
        out += res.results[c]["energy"]
    return out


# revision 20
# speedup vs baseline: 1.0885x; 1.0885x over previous
"""AlchemicalGAT Trainium2 kernel (8 NeuronCores, SPMD).

Strategy:
  - Shard dst nodes contiguously across 8 cores (2500 each); edges sharded by dst.
  - Per layer, each core builds a packed per-node staging row in SBUF:
    [h (4x64 fp8e4m3, 256B) | esrc hi/lo (bf16x4 each)] = 272B, plus a tiny
    per-node edst row [ed hi/lo] = 16B kept locally (dst nodes are owned).
  - The staging is exchanged in TWO half-slot AllGathers (blocks 0-9, 10-19)
    so the second collective overlaps the first half's edge processing; each
    AllGather result is repitched to a 512B-row DRAM gather table.
  - Edge phase runs two passes per layer: pass A (edges whose src lives in
    the first half-slot) accumulates partial num/den into SBUF right after
    AllGather A; pass B finishes after AllGather B. Per dst block: one
    dma_gather of 512B rows by src (h+esrc), one 256B gather by dst from the
    local edst table; softmax without max-subtraction; segment-sum via
    one-hot matmul into PSUM.
  - LayerNorm folded into the conv1 matmul: x^T arrives via DMA transpose,
    mean/sumsq come from ones-matmuls, and [mu; 1/r] form two extra lhsT rows
    against host-built correction weights; the result is scaled by r per node.
  - Node transforms/MLP batch channel pairs via block-diagonal bf16 weights.
  - Per-structure energies via matmul-masked segment sum; host sums 8 cores.
"""
import sys, os
sys.path.insert(0, "/opt/trn_rl_repo")
import math
import numpy as np
import ml_dtypes
from contextlib import ExitStack

import concourse.bass as bass
import concourse.bacc as bacc
import concourse.mybir as mybir
import concourse.tile as tile
from concourse.bass_utils import run_bass_kernel_spmd
from concourse.masks import make_identity

F32 = mybir.dt.float32
BF16 = mybir.dt.bfloat16
U8 = mybir.dt.uint8
FP8 = mybir.dt.float8e4
I16 = mybir.dt.int16
AF = mybir.ActivationFunctionType
ALU = mybir.AluOpType

N = 20000
E = 400000
C = 4
F = 128
O = 64          # conv1/conv2 width
H1, H2 = 64, 32
S = 32          # structures
NCORE = 8
NPC = N // NCORE            # 2500 nodes per core
BLK = 128
NBLK = math.ceil(NPC / BLK)  # 20 (last block 68 nodes)
LAST = NPC - (NBLK - 1) * BLK  # 68
SLOT = NBLK * BLK            # 2560 staging rows per core
NBH = NBLK // 2              # blocks per half
HALF = NBH * BLK             # 1280 rows per half-slot
SROW = 272                   # staging row bytes (h fp8 256 + esrc hi/lo)
EROW = 16                    # local edst row bytes (ed hi/lo)
LROW = 256                   # local edst DRAM table pitch
TROW = 512                   # DRAM gather table row pitch bytes
NEG = -1.0e30

GCHUNK = int(os.environ.get("GAT_GCHUNK", "8"))  # gather chunk (tiles)


# ---------------------------------------------------------------- host side

def _wrap_idx16(ids):
    """[n] int -> dma_gather index layout [128, n//16] int16 (q -> [q%16, q//16],
    replicated over the 8 Q7 cores)."""
    n = ids.shape[0]
    assert n % 16 == 0
    out = np.zeros((16, n // 16), np.int16)
    q = np.arange(n)
    out[q % 16, q // 16] = ids.astype(np.int16)
    return np.tile(out, (8, 1))


def _prep(inputs):
    x = np.asarray(inputs["x"], dtype=np.float32)
    ei = np.asarray(inputs["edge_index"]).astype(np.int64)
    bid = np.asarray(inputs["batch_ids"]).astype(np.int64)
    gamma = np.asarray(inputs["gamma"], np.float32)
    beta = np.asarray(inputs["beta"], np.float32)
    src, dst = ei[0], ei[1]

    order = np.argsort(dst, kind="stable")
    src_s, dst_s = src[order], dst[order]

    counts = np.bincount(dst_s, minlength=N)
    starts = np.concatenate([[0], np.cumsum(counts)])

    # split each (core, block) edge list by src half-slot (src%NPC < HALF)
    eA, eB = {}, {}
    cA = np.zeros((NCORE, NBLK), np.int64)
    cB = np.zeros((NCORE, NBLK), np.int64)
    for c in range(NCORE):
        for b in range(NBLK):
            lo = c * NPC + b * BLK
            hi = min(c * NPC + min((b + 1) * BLK, NPC), (c + 1) * NPC)
            e0, e1 = starts[lo], starts[hi]
            ss, dd = src_s[e0:e1], dst_s[e0:e1]
            isA = (ss % NPC) < HALF
            eA[c, b] = (ss[isA], dd[isA])
            eB[c, b] = (ss[~isA], dd[~isA])
            cA[c, b], cB[c, b] = isA.sum(), (~isA).sum()
    tA = tuple(int(math.ceil(cA[:, b].max() / BLK)) for b in range(NBLK))
    tB = tuple(int(math.ceil(cB[:, b].max() / BLK)) for b in range(NBLK))
    assert all(t > 0 for t in tA) and all(t > 0 for t in tB)
    # tile layout per block: [A tiles | B tiles], concatenated over blocks
    offA, offB = [], []
    o = 0
    for b in range(NBLK):
        offA.append(o)
        o += tA[b]
        offB.append(o)
        o += tB[b]
    TT = o

    per_core = []
    for c in range(NCORE):
        s16 = np.zeros((128, TT * 8), np.int16)
        d16 = np.zeros((128, TT * 8), np.int16)
        dl = np.full((128, TT, 1), 127.0, np.float32)

        def fill(b, off, nt, ss, dd, half_b):
            EBn = nt * BLK
            sid = np.zeros(EBn, np.int64)        # pad -> row 0 (killed via dst)
            did = np.full(EBn, NPC, np.int64)    # pad -> localE pad row
            dloc = np.full(EBn, 127, np.int64)
            n = len(ss)
            r = ss % NPC
            sid[:n] = (ss // NPC) * HALF + (r - HALF if half_b else r)
            did[:n] = dd - c * NPC
            dloc[:n] = dd - (c * NPC + b * BLK)
            s16[:, off * 8:(off + nt) * 8] = _wrap_idx16(sid)
            d16[:, off * 8:(off + nt) * 8] = _wrap_idx16(did)
            dl[:, off:off + nt, 0] = dloc.reshape(nt, 128).T

        for b in range(NBLK):
            fill(b, offA[b], tA[b], *eA[c, b], False)
            fill(b, offB[b], tB[b], *eB[c, b], True)
        # structure mask [128, NBLK*S]
        bm = np.zeros((128, NBLK * S), np.float32)
        for b in range(NBLK):
            cnt = BLK if b < NBLK - 1 else LAST
            g = c * NPC + b * BLK + np.arange(cnt)
            bm[np.arange(cnt), b * S + bid[g]] = 1.0
        xs = np.zeros((NBLK * BLK, C * F), np.float32)
        xs[:NPC] = x[c * NPC:(c + 1) * NPC].reshape(NPC, C * F)
        per_core.append(dict(
            xs=xs.astype(ml_dtypes.bfloat16),
            src16=s16, dst16=d16,
            dstloc=dl.astype(ml_dtypes.bfloat16),
            bmask=bm.astype(ml_dtypes.bfloat16)))

    # ---- weights
    bg = beta / np.where(gamma == 0, 1.0, gamma)

    def wcat(W, a_s, a_d):
        out = np.zeros((C, W.shape[1], 66), np.float32)
        out[:, :, :O] = W
        out[:, :, O] = np.einsum("cfo,co->cf", W, a_s)
        out[:, :, O + 1] = np.einsum("cfo,co->cf", W, a_d)
        return out

    Wc1 = np.asarray(inputs["Wc1"], np.float32) * gamma[None, :, None]
    wcat1 = wcat(Wc1, np.asarray(inputs["asrc1"], np.float32),
                 np.asarray(inputs["adst1"], np.float32))       # [C,F,66]
    wcat2 = wcat(np.asarray(inputs["Wc2"], np.float32),
                 np.asarray(inputs["asrc2"], np.float32),
                 np.asarray(inputs["adst2"], np.float32))       # [C,O,66]

    # conv1 rhs [F, C*66]
    w1 = np.zeros((F, C * 66), np.float32)
    for c in range(C):
        w1[:, c * 66:(c + 1) * 66] = wcat1[c]
    # LN-correction rhs [8, C*66]: row c = -colsum_f(wcat1_c); row 4+c =
    # sum_f bg[f]*wcat1_c[f,:]
    rhs2 = np.zeros((8, C * 66), np.float32)
    for c in range(C):
        rhs2[c, c * 66:(c + 1) * 66] = -wcat1[c].sum(axis=0)
        rhs2[4 + c, c * 66:(c + 1) * 66] = bg @ wcat1[c]

    # conv2 block-diag pairs [O*2=128, 2*132]
    w2bd = np.zeros((128, 2 * 132), np.float32)
    for p in range(2):
        for j in range(2):
            c = 2 * p + j
            w2bd[j * O:(j + 1) * O, p * 132 + j * 66:p * 132 + (j + 1) * 66] = wcat2[c]

    Wn1 = np.asarray(inputs["Wn1"], np.float32)
    Wn2 = np.asarray(inputs["Wn2"], np.float32)
    Wout = np.asarray(inputs["Wout"], np.float32) / np.float32(math.sqrt(C) * 20.0)
    wn1bd = np.zeros((128, 2 * 128), np.float32)
    wn2bd = np.zeros((128, 2 * 64), np.float32)
    for p in range(2):
        for j in range(2):
            c = 2 * p + j
            wn1bd[j * O:(j + 1) * O, p * 128 + j * O:p * 128 + (j + 1) * O] = Wn1[c]
            wn2bd[j * O:(j + 1) * O, p * 64 + j * H2:p * 64 + (j + 1) * H2] = Wn2[c]
    wof = np.zeros((128, 1), np.float32)
    for c in range(C):
        wof[c * H2:(c + 1) * H2, 0] = Wout[c, :, 0]

    iota = np.tile(np.arange(128, dtype=np.float32), (128, 1)).reshape(128, 1, 128)
    oneF = np.full((128, 1), 1.0 / F, np.float32)
    padmask = np.ones((128, 1), np.float32); padmask[LAST, 0] = 0.0
    padneg = np.zeros((128, 1), np.float32); padneg[LAST, 0] = NEG

    b16 = lambda a: np.asarray(a, np.float32).astype(ml_dtypes.bfloat16)
    shared = dict(
        w1=b16(w1), rhs2=b16(rhs2), w2bd=b16(w2bd),
        wn1bd=b16(wn1bd), wn2bd=b16(wn2bd), wof=b16(wof),
        iota=b16(iota), oneF=b16(oneF),
        padmask=padmask, padneg=padneg,
    )
    return per_core, shared, (tA, tB)


# ---------------------------------------------------------------- device side

STOP_AFTER = os.environ.get("GAT_STOP_AFTER", "")  # node1|ag1|edge1|node2|edge2


def _build(tkey):
    tA, tB = [list(t) for t in tkey]
    nc = bacc.Bacc("TRN2", target_bir_lowering=False, debug=False,
                   enable_asserts=False, num_devices=NCORE)
    offA, offB = [], []
    o = 0
    for b in range(NBLK):
        offA.append(o)
        o += tA[b]
        offB.append(o)
        o += tB[b]
    TT = o
    tmaxA, tmaxB = max(tA), max(tB)

    xs_d = nc.dram_tensor("xs", [NBLK * BLK, C * F], BF16, kind="ExternalInput")
    s16_d = nc.dram_tensor("src16", [128, TT * 8], I16, kind="ExternalInput")
    d16_d = nc.dram_tensor("dst16", [128, TT * 8], I16, kind="ExternalInput")
    dl_d = nc.dram_tensor("dstloc", [128, TT, 1], BF16, kind="ExternalInput")
    bm_d = nc.dram_tensor("bmask", [128, NBLK * S], BF16, kind="ExternalInput")
    w1_d = nc.dram_tensor("w1", [F, C * 66], BF16, kind="ExternalInput")
    rhs2_d = nc.dram_tensor("rhs2", [8, C * 66], BF16, kind="ExternalInput")
    w2bd_d = nc.dram_tensor("w2bd", [128, 2 * 132], BF16, kind="ExternalInput")
    wn1bd_d = nc.dram_tensor("wn1bd", [128, 2 * 128], BF16, kind="ExternalInput")
    wn2bd_d = nc.dram_tensor("wn2bd", [128, 2 * 64], BF16, kind="ExternalInput")
    wof_d = nc.dram_tensor("wof", [128, 1], BF16, kind="ExternalInput")
    io_d = nc.dram_tensor("iota", [128, 1, 128], BF16, kind="ExternalInput")
    one_d = nc.dram_tensor("oneF", [128, 1], BF16, kind="ExternalInput")
    pm_d = nc.dram_tensor("padmask", [128, 1], F32, kind="ExternalInput")
    pn_d = nc.dram_tensor("padneg", [128, 1], F32, kind="ExternalInput")
    out_d = nc.dram_tensor("energy", [S, 1], F32, kind="ExternalOutput")

    with tile.TileContext(nc, num_cores=NCORE) as tc, ExitStack() as ctx:
        per = ctx.enter_context(tc.tile_pool(name="persist", bufs=1))
        sbw = ctx.enter_context(tc.tile_pool(
            name="work", bufs=int(os.environ.get("GAT_SBW_BUFS", "2"))))
        sbn = ctx.enter_context(tc.tile_pool(
            name="nwork", bufs=int(os.environ.get("GAT_SBN_BUFS", "3"))))
        ps_node = ctx.enter_context(tc.tile_pool(
            name="psn", bufs=int(os.environ.get("GAT_PSN_BUFS", "2")), space="PSUM"))
        ps_tp = ctx.enter_context(tc.tile_pool(name="pst", bufs=2, space="PSUM"))
        ps_edge = ctx.enter_context(tc.tile_pool(
            name="pse", bufs=int(os.environ.get("GAT_PSE_BUFS", "2")), space="PSUM"))
        dram = ctx.enter_context(tc.tile_pool(name="dram", bufs=1, space="DRAM"))

        # persistent tiles
        s16 = per.tile([128, TT * 8], I16)
        d16 = per.tile([128, TT * 8], I16)
        dl = per.tile([128, TT, 1], BF16)
        bm = per.tile([128, NBLK * S], BF16)
        iot = per.tile([128, 1, 128], BF16)
        oneF = per.tile([128, 1], BF16)
        w1 = per.tile([F, C * 66], BF16)
        rhs2 = per.tile([8, C * 66], BF16)
        w2bd = per.tile([128, 2 * 132], BF16)
        wn1bd = per.tile([128, 2 * 128], BF16)
        wn2bd = per.tile([128, 2 * 64], BF16)
        wof = per.tile([128, 1], BF16)
        pmsk = per.tile([128, 1], F32)
        pneg = per.tile([128, 1], F32)
        ident = per.tile([128, 128], BF16)
        stg = per.tile([128, NBLK * SROW], U8)
        stgE = per.tile([128, NBLK * EROW], U8)
        NumDen = per.tile([128, NBLK, 260], F32)
        Hb = per.tile([128, NBLK, C * O], BF16)
        H3 = per.tile([128, NBLK, C * O], BF16)
        Eb = per.tile([128, NBLK], BF16)
        eps = per.tile([128, 1], F32)
        nc.vector.memset(eps[:], 1e-5)

        nc.sync.dma_start(s16[:], s16_d[:, :])
        nc.sync.dma_start(d16[:], d16_d[:, :])
        nc.sync.dma_start(dl[:], dl_d[:, :, :])
        nc.sync.dma_start(bm[:], bm_d[:, :])
        nc.sync.dma_start(iot[:], io_d[:, :, :])
        nc.sync.dma_start(oneF[:], one_d[:, :])
        nc.sync.dma_start(w1[:], w1_d[:, :])
        nc.sync.dma_start(rhs2[:], rhs2_d[:, :])
        nc.sync.dma_start(w2bd[:], w2bd_d[:, :])
        nc.sync.dma_start(wn1bd[:], wn1bd_d[:, :])
        nc.sync.dma_start(wn2bd[:], wn2bd_d[:, :])
        nc.sync.dma_start(wof[:], wof_d[:, :])
        nc.sync.dma_start(pmsk[:], pm_d[:, :])
        nc.sync.dma_start(pneg[:], pn_d[:, :])
        make_identity(nc, ident[:])

        # DRAM: packed staging halves, AllGather outputs, 512B-pitch gather
        # tables (per layer, per half) + local edst tables
        tbsrc = [[dram.tile([HALF, SROW], U8, name=f"tbsrc{i}{h}")
                  for h in range(2)] for i in range(2)]
        agd = [[dram.tile([NCORE * HALF, SROW], U8, name=f"agd{i}{h}")
                for h in range(2)] for i in range(2)]
        tbl = [[dram.tile([NCORE * HALF, TROW], U8, name=f"tbl{i}{h}")
                for h in range(2)] for i in range(2)]
        localE = [dram.tile([SLOT, LROW], U8, name=f"localE{i}") for i in range(2)]

        def stg_views(nt):
            blk = stg[:, nt * SROW:(nt + 1) * SROW]
            h = blk[:, 0:256].bitcast(FP8).rearrange("p (c u) -> p c u", c=C)
            es_hi = blk[:, 256:264].bitcast(BF16)
            es_lo = blk[:, 264:272].bitcast(BF16)
            eb = stgE[:, nt * EROW:(nt + 1) * EROW]
            ed_hi = eb[:, 0:8].bitcast(BF16)
            ed_lo = eb[:, 8:16].bitcast(BF16)
            return h, es_hi, es_lo, ed_hi, ed_lo

        def assemble(nt, hsrc, esrc_f32, edst_f32):
            """hsrc [128,C,64] -> fp8; esrc/edst f32 [128,4] -> bf16 hi/lo."""
            h, es_hi, es_lo, ed_hi, ed_lo = stg_views(nt)
            nc.vector.tensor_copy(h[:], hsrc)
            nc.vector.tensor_copy(es_hi[:], esrc_f32)
            nc.vector.tensor_tensor(out=es_lo[:], in0=esrc_f32, in1=es_hi[:],
                                    op=ALU.subtract)
            nc.vector.tensor_copy(ed_hi[:], edst_f32)
            nc.vector.tensor_tensor(out=ed_lo[:], in0=edst_f32, in1=ed_hi[:],
                                    op=ALU.subtract)
            if nt == NBLK - 1:
                # pad row (local row NPC = row LAST of the last block): force
                # ed_hi = NEG there via full-partition masked ops so every pad
                # edge (which points its dst at this row) gets ex = 0.
                nc.vector.tensor_scalar(out=ed_hi[:], in0=ed_hi[:],
                                        scalar1=pmsk[:], scalar2=None,
                                        op0=ALU.mult)
                nc.vector.tensor_scalar(out=ed_hi[:], in0=ed_hi[:],
                                        scalar1=pneg[:], scalar2=None,
                                        op0=ALU.add)

        def exchange_half(layer, h):
            src = stg[:, h * NBH * SROW:(h + 1) * NBH * SROW]
            nc.sync.dma_start(
                tbsrc[layer][h][:, :].rearrange("(b p) c -> p b c", p=128),
                src.rearrange("p (b c) -> p b c", b=NBH))
            nc.gpsimd.collective_compute(
                "AllGather", ALU.bypass, replica_groups=[list(range(NCORE))],
                ins=[tbsrc[layer][h][:, :]], outs=[agd[layer][h][:, :]])
            nc.sync.dma_start(tbl[layer][h][:, 0:SROW], agd[layer][h][:, :])

        def write_localE(layer):
            nc.sync.dma_start(
                localE[layer][:, 0:EROW].rearrange("(b p) c -> p b c", p=128),
                stgE[:, :].rearrange("p (b c) -> p b c", b=NBLK))

        def node_phase1(do_ag=True):
            for nt in range(NBLK):
                # x^T (all channels) via one DMA transpose: [128 f, C, 128 n]
                xT = sbn.tile([128, C, 128], BF16, tag="xT")
                nc.sync.dma_start_transpose(
                    xT[:], xs_d[nt * BLK:(nt + 1) * BLK, :])
                xsq = sbn.tile([128, C, 128], BF16, tag="xsq")
                nc.scalar.activation(xsq[:], xT[:], AF.Square)
                # stats: mu_c, sumsq_c/F via ones-matmuls -> [128 nodes, 8]
                stp = ps_tp.tile([128, 8], F32, space="PSUM", tag="tp")
                for cc in range(C):
                    nc.tensor.matmul(out=stp[:, cc:cc + 1],
                                     lhsT=xT[:, cc, :],
                                     rhs=oneF[:, :], start=True, stop=True)
                    nc.tensor.matmul(out=stp[:, 4 + cc:5 + cc],
                                     lhsT=xsq[:, cc, :],
                                     rhs=oneF[:, :], start=True, stop=True)
                sb8 = sbn.tile([128, 8], BF16, tag="sb8")
                nc.vector.tensor_copy(sb8[:, 0:4], stp[:, 0:4])
                mu2 = sbn.tile([128, 4], F32, tag="mu2")
                nc.vector.tensor_tensor(out=mu2[:], in0=sb8[:, 0:4],
                                        in1=sb8[:, 0:4], op=ALU.mult)
                var = sbn.tile([128, 4], F32, tag="var")
                nc.vector.tensor_tensor(out=var[:], in0=stp[:, 4:8],
                                        in1=mu2[:], op=ALU.subtract)
                invr = sbn.tile([128, 4], F32, tag="invr")
                nc.scalar.activation(invr[:], var[:], AF.Sqrt, bias=eps[:])
                rr = sbn.tile([128, 4], F32, tag="rr")
                nc.vector.reciprocal(rr[:], invr[:])
                nc.vector.tensor_copy(sb8[:, 4:8], invr[:])
                # [mu | invr] -> transpose -> [8, 128] lhsT rows
                tp8 = ps_tp.tile([8, 128], BF16, space="PSUM", tag="tp")
                nc.tensor.transpose(out=tp8[:], in_=sb8[:], identity=ident[:])
                st8 = sbn.tile([8, 128], BF16, tag="st8")
                nc.vector.tensor_copy(st8[:], tp8[:])
                # conv1 matmul + LN correction rows, then scale by r
                nps = ps_node.tile([128, C * 66], F32, space="PSUM", tag="nps")
                for cc in range(C):
                    nc.tensor.matmul(out=nps[:, cc * 66:(cc + 1) * 66],
                                     lhsT=xT[:, cc, :],
                                     rhs=w1[:, cc * 66:(cc + 1) * 66],
                                     start=True, stop=False)
                    nc.tensor.matmul(out=nps[:, cc * 66:(cc + 1) * 66],
                                     lhsT=st8[:, :],
                                     rhs=rhs2[:, cc * 66:(cc + 1) * 66],
                                     start=False, stop=True)
                ht = sbn.tile([128, C * 66], F32, tag="ht")
                for cc in range(C):
                    nc.vector.tensor_scalar_mul(
                        ht[:, cc * 66:(cc + 1) * 66],
                        nps[:, cc * 66:(cc + 1) * 66], rr[:, cc:cc + 1])
                htv = ht[:, :].rearrange("p (c u) -> p c u", c=C)
                assemble(nt, htv[:, :, 0:64], htv[:, :, 64], htv[:, :, 65])
                if do_ag and nt == NBH - 1:
                    exchange_half(0, 0)
            if do_ag:
                exchange_half(0, 1)
                write_localE(0)

        def node_phase2(do_ag=True):
            for nt in range(NBLK):
                nps = ps_node.tile([128, 264], F32, space="PSUM", tag="nps")
                hT = sbn.tile([128, 2, 128], BF16, tag="hT")
                nc.sync.dma_start_transpose(hT[:], Hb[:, nt, :])
                for p in range(2):
                    nc.tensor.matmul(out=nps[:, p * 132:(p + 1) * 132],
                                     lhsT=hT[:, p, :],
                                     rhs=w2bd[:, p * 132:(p + 1) * 132],
                                     start=True, stop=True)
                h, es_hi, es_lo, ed_hi, ed_lo = stg_views(nt)
                es = sbn.tile([128, 4], F32, tag="es")
                ed = sbn.tile([128, 4], F32, tag="ed")
                for p in range(2):
                    v = nps[:, p * 132:(p + 1) * 132].rearrange(
                        "p (c u) -> p c u", c=2)
                    nc.vector.tensor_copy(h[:, 2 * p:2 * p + 2, :], v[:, :, 0:64])
                    nc.vector.tensor_copy(es[:, 2 * p:2 * p + 2], v[:, :, 64])
                    nc.vector.tensor_copy(ed[:, 2 * p:2 * p + 2], v[:, :, 65])
                nc.vector.tensor_copy(es_hi[:], es[:])
                nc.vector.tensor_tensor(out=es_lo[:], in0=es[:], in1=es_hi[:],
                                        op=ALU.subtract)
                nc.vector.tensor_copy(ed_hi[:], ed[:])
                nc.vector.tensor_tensor(out=ed_lo[:], in0=ed[:], in1=ed_hi[:],
                                        op=ALU.subtract)
                if nt == NBLK - 1:
                    nc.vector.tensor_scalar(out=ed_hi[:], in0=ed_hi[:],
                                            scalar1=pmsk[:], scalar2=None,
                                            op0=ALU.mult)
                    nc.vector.tensor_scalar(out=ed_hi[:], in0=ed_hi[:],
                                            scalar1=pneg[:], scalar2=None,
                                            op0=ALU.add)
                if do_ag and nt == NBH - 1:
                    exchange_half(1, 0)
            if do_ag:
                exchange_half(1, 1)
                write_localE(1)

        def edge_pass(layer, half, Hout):
            """half 0 (A): accumulate partial num/den into NumDen.
            half 1 (B): add NumDen, normalize, silu -> Hout."""
            toff = offA if half == 0 else offB
            tcnt = tA if half == 0 else tB
            tmx = tmaxA if half == 0 else tmaxB
            for b in range(NBLK):
                tb = tcnt[b]
                off = toff[b]
                G = sbw.tile([128, tmx, TROW], U8, tag=f"G{half}")
                D = sbw.tile([128, tmx, 256], U8, tag=f"D{half}")
                for t0 in range(0, tb, GCHUNK):
                    t1 = min(t0 + GCHUNK, tb)
                    k = (t1 - t0) * BLK
                    o0, o1 = (off + t0) * 8, (off + t1) * 8
                    nc.gpsimd.dma_gather(G[:, t0:t1, :], tbl[layer][half][:, :],
                                         s16[:, o0:o1], k, k, TROW)
                    nc.gpsimd.dma_gather(D[:, t0:t1, :], localE[layer][:, :],
                                         d16[:, o0:o1], k, k, 256)
                OS = sbw.tile([128, tmx, 128], BF16, tag=f"OS{half}")
                nc.vector.tensor_tensor(
                    out=OS[:, 0:tb],
                    in0=dl[:, off:off + tb, :].to_broadcast([128, tb, 128]),
                    in1=iot[:, :, :].to_broadcast([128, tb, 128]),
                    op=ALU.is_equal)
                gs_hi = G[:, :, 256:264].bitcast(BF16)
                gs_lo = G[:, :, 264:272].bitcast(BF16)
                dd_hi = D[:, :, 0:8].bitcast(BF16)
                dd_lo = D[:, :, 8:16].bitcast(BF16)
                EX = sbw.tile([128, tmx, C], F32, tag=f"EX{half}")
                nc.vector.tensor_tensor(out=EX[:, 0:tb], in0=gs_hi[:, 0:tb],
                                        in1=gs_lo[:, 0:tb], op=ALU.add)
                nc.vector.tensor_tensor(out=EX[:, 0:tb], in0=EX[:, 0:tb],
                                        in1=dd_hi[:, 0:tb], op=ALU.add)
                nc.vector.tensor_tensor(out=EX[:, 0:tb], in0=EX[:, 0:tb],
                                        in1=dd_lo[:, 0:tb], op=ALU.add)
                nc.vector.scalar_tensor_tensor(
                    out=EX[:, 0:tb], in0=EX[:, 0:tb], scalar=0.2,
                    in1=EX[:, 0:tb], op0=ALU.mult, op1=ALU.max)
                nc.scalar.activation(EX[:, 0:tb], EX[:, 0:tb], AF.Exp)
                Gh = G[:, :, 0:256].bitcast(FP8).rearrange(
                    "p t (c u) -> p t c u", c=C)
                Gw = sbw.tile([128, tmx, 260], BF16, tag=f"Gw{half}")
                Gwv = Gw[:, :, 0:256].rearrange("p t (c u) -> p t c u", c=C)
                nc.vector.tensor_tensor(
                    out=Gwv[:, 0:tb], in0=Gh[:, 0:tb],
                    in1=EX[:, 0:tb].to_broadcast([128, tb, C, 64]), op=ALU.mult)
                nc.vector.tensor_copy(Gw[:, 0:tb, 256:260], EX[:, 0:tb])
                ps = ps_edge.tile([128, 260], F32, space="PSUM", tag="ep")
                for t in range(tb):
                    nc.tensor.matmul(out=ps[:], lhsT=OS[:, t, :], rhs=Gw[:, t, :],
                                     start=(t == 0), stop=(t == tb - 1))
                if half == 0:
                    nc.vector.tensor_copy(NumDen[:, b, :], ps[:])
                    continue
                tot = sbw.tile([128, 260], F32, tag="tot")
                nc.vector.tensor_tensor(out=tot[:], in0=ps[:],
                                        in1=NumDen[:, b, :], op=ALU.add)
                dn = sbw.tile([128, C], F32, tag="dn")
                nc.vector.tensor_scalar(out=dn[:], in0=tot[:, 256:260],
                                        scalar1=1e-16, scalar2=None, op0=ALU.add)
                rc = sbw.tile([128, C], F32, tag="rc")
                nc.vector.reciprocal(rc[:], dn[:])
                om = sbw.tile([128, C * O], BF16, tag="om")
                omv = om[:, :].rearrange("p (c u) -> p c u", c=C)
                totv = tot[:, 0:256].rearrange("p (c u) -> p c u", c=C)
                nc.vector.tensor_tensor(out=omv[:, :, :], in0=totv[:, :, :],
                                        in1=rc[:].to_broadcast([128, C, 64]),
                                        op=ALU.mult)
                nc.scalar.activation(Hout[:, b, :], om[:], AF.Silu)

        def edge_phase(layer, Hout):
            edge_pass(layer, 0, None)
            edge_pass(layer, 1, Hout)

        phases = {"node1": 1, "ag1": 2, "edge1": 3, "node2": 4, "edge2": 5}
        stop = phases.get(STOP_AFTER, 99)

        node_phase1(do_ag=(stop >= 2))
        if stop >= 3:
            edge_phase(0, Hb)
        if stop >= 4:
            node_phase2(do_ag=(stop >= 5))
        if stop >= 5:
            edge_phase(1, H3)
        do_tail = stop >= 6
        if not do_tail:
            eo0 = sbn.tile([S, 1], F32, tag="eo")
            nc.vector.memset(eo0[:], 0.0)
            nc.sync.dma_start(out_d[:, :], eo0[:])

        # MLP + channel sum + structure segment sum
        for nt in (range(NBLK) if do_tail else []):
            u2st = sbn.tile([128, 128], BF16, tag="u2st")
            p1 = ps_node.tile([128, 264], F32, space="PSUM", tag="nps")
            hT = sbn.tile([128, 2, 128], BF16, tag="tT")
            nc.sync.dma_start_transpose(hT[:], H3[:, nt, :])
            for p in range(2):
                nc.tensor.matmul(out=p1[:, p * 128:(p + 1) * 128],
                                 lhsT=wn1bd[:, p * 128:(p + 1) * 128],
                                 rhs=hT[:, p, :], start=True, stop=True)
            for p in range(2):
                u1 = sbn.tile([128, 128], BF16, tag=f"u1{p}")
                nc.scalar.activation(u1[:], p1[:, p * 128:(p + 1) * 128], AF.Silu)
                p2 = ps_edge.tile([64, 128], F32, space="PSUM", tag="ep")
                nc.tensor.matmul(out=p2[:], lhsT=wn2bd[:, p * 64:(p + 1) * 64],
                                 rhs=u1[:], start=True, stop=True)
                nc.scalar.activation(u2st[p * 64:(p + 1) * 64, :], p2[:], AF.Silu)
            p3 = ps_edge.tile([128, 1], F32, space="PSUM", tag="ep")
            nc.tensor.matmul(out=p3[:], lhsT=u2st[:], rhs=wof[:, :],
                             start=True, stop=True)
            nc.vector.tensor_copy(Eb[:, nt:nt + 1], p3[:])
        if do_tail:
            psS = ps_tp.tile([S, 1], F32, space="PSUM", tag="tp")
            for nt in range(NBLK):
                nc.tensor.matmul(out=psS[:], lhsT=bm[:, nt * S:(nt + 1) * S],
                                 rhs=Eb[:, nt:nt + 1],
                                 start=(nt == 0), stop=(nt == NBLK - 1))
            eo = sbn.tile([S, 1], F32, tag="eo")
            nc.vector.tensor_copy(eo[:], psS[:])
            nc.sync.dma_start(out_d[:, :], eo[:])
    nc.compile()
    return nc


_CACHE = {}


def kernel(**inputs):
    per_core, shared, tkey = _prep(inputs)
    if tkey not in _CACHE:
        _CACHE[tkey] = _build(tkey)
    nc = _CACHE[tkey]
    in_maps = []
    for c in range(NCORE):
        pc = per_core[c]
        m = {"xs": pc["xs"], "src16": pc["src16"], "dst16": pc["dst16"],
             "dstloc": pc["dstloc"], "bmask": pc["bmask"]}
        m.update(shared)
        in_maps.append(m)
    res = run_bass_kernel_spmd(nc, in_maps, core_ids=list(range(NCORE)))
    out = np.zeros((S, 1), np.float32)
    for c in range(NCORE):
        out += res.results[c]["energy"]
    return out
